# revision 1
# baseline (speedup 1.0000x reference)
"""ClashLoss kernel for Trainium2 (8 NeuronCores, batch-parallel).

Math: for each batch b, count pairs (n, m), n != m, with
    dist(n, m) < radii[n] + radii[m]   and   dist(n, m) > EPS.
Using s_n = |c_n|^2 - r_n^2, the clash condition dist^2 < (r_n + r_m)^2 is
    G[n, m] = dot(c_n, c_m) + r_n r_m - s_n/2 - s_m/2 > 0,
i.e. a 6-dim dot product u_n . v_m with
    u_n = (x, y, z, r_n, -s_n/2, 1)     (matmul stationary side)
    v_m = (x, y, z, r_m, 1, -s_m/2)     (matmul moving side)
The PE computes G tile-by-tile into PSUM; ACT (Sign + accumulate) and DVE
(tensor_scalar is_gt + accumulate) count positives per 512x512 super-block.
Symmetry: only upper-triangular super-blocks are computed (weight 2), the
diagonal super-blocks once (weight 1).  The diagonal n == m has
G[n,n] = 2 r_n^2 > 0 always, so exactly N diagonal hits are subtracted on
the host.

Raw-bass implementation (not Tile): fp32 matmuls only support a single
sync-wait in codegen, so semaphores are placed by hand -- at most one wait
per matmul, standalone wait instructions elsewhere.
"""

import numpy as np

N = 4096
B = 8
K = 6  # augmented dot-product length
SUPER = 512  # super-block edge (4 PSUM banks wide)
NSUP = N // SUPER  # 8
ROWT = 128  # rows per matmul (output partitions)
CHUNK_FD = SUPER * 2  # 1024 f32 = 2 PSUM banks; 4 chunks rotate
NCHUNKS = 4
MM_PER_UNIT = 2  # each unit = one chunk = 2 row-tile matmuls
UNITS_PER_SUPER = 2
EPS = 1e-8

# super-block schedule: (R, C, weight); R <= C
SCHEDULE = [(r, c, 1 if r == c else 2) for r in range(NSUP) for c in range(r, NSUP)]
NSLOTS = len(SCHEDULE)  # 36

# engine assignment: interleave ACT/DVE (Bresenham spread) so the two
# consumer engines run concurrently on alternating PSUM chunk buffers;
# ACT is a bit faster per chunk so it takes 20 of 36 supers.
NUNITS = NSLOTS * UNITS_PER_SUPER  # 72
N_ACT_TARGET = 40
ENGINE = [
    "act"
    if (i + 1) * N_ACT_TARGET // NUNITS > i * N_ACT_TARGET // NUNITS
    else "dve"
    for i in range(NUNITS)
]
# compact per-engine slot index for each unit
SLOT_IDX = []
_na = _nd = 0
for _i in range(NUNITS):
    if ENGINE[_i] == "act":
        SLOT_IDX.append(_na)
        _na += 1
    else:
        SLOT_IDX.append(_nd)
        _nd += 1
N_ACT, N_DVE = _na, _nd

_CACHE = {}


def _build(repeat=1, mm_dtype="float32r"):
    """Build the raw-bass SPMD program (same program for all cores).

    repeat > 1 re-runs the whole super-block schedule (for differential
    wall-clock timing); the counts are simply overwritten each pass.
    """
    import concourse.bass as bass
    from concourse import mybir

    nc = bass.Bass("TRN2", target_bir_lowering=False, debug=False)
    f32 = mybir.dt.float32
    mdt = getattr(mybir.dt, mm_dtype)

    u_dram = nc.dram_tensor("u6", [K, N], mdt, kind="ExternalInput").ap()
    v_dram = nc.dram_tensor("v6", [K, N], mdt, kind="ExternalInput").ap()
    out_dram = nc.dram_tensor(
        "counts", [128, N_ACT + N_DVE], f32, kind="ExternalOutput"
    ).ap()

    # flattened unit schedule over repeats: unit -> (R, C, half)
    usched = []
    for R, C, _w in SCHEDULE:
        usched.append((R, C, 0))
        usched.append((R, C, 1))
    gsched = usched * repeat
    gengine = ENGINE * repeat
    gslot = SLOT_IDX * repeat
    ntot = len(gsched)

    # consumer bookkeeping: for global super g, which engine consumes it and
    # the cumulative per-engine consumption count up to and including g.
    cons_count = []  # (engine, count_after_g)
    na = nd = 0
    for g in range(ntot):
        if gengine[g] == "act":
            na += 1
            cons_count.append(("act", na))
        else:
            nd += 1
            cons_count.append(("dve", nd))
    n_act_total, n_dve_total = na, nd

    with (
        nc.sbuf_tensor([K, N], mdt) as u_sb,
        nc.sbuf_tensor([K, N], mdt) as v_sb,
        nc.sbuf_tensor([128, max(1, N_ACT)], f32) as act_slots,
        nc.sbuf_tensor([128, max(1, N_DVE)], f32) as dve_slots,
        nc.sbuf_tensor([128, NUNITS + 3], f32) as act_dummy,
        nc.sbuf_tensor([128, NUNITS], f32) as dve_dummy,
        nc.psum_tensor([128, CHUNK_FD], f32) as chunk0,
        nc.psum_tensor([128, CHUNK_FD], f32) as chunk1,
        nc.psum_tensor([128, CHUNK_FD], f32) as chunk2,
        nc.psum_tensor([128, CHUNK_FD], f32) as chunk3,
        nc.semaphore("DMA_IN") as s_in,
        nc.semaphore("PROD") as s_prod,
        nc.semaphore("CACT") as s_cact,
        nc.semaphore("CDVE") as s_cdve,
        nc.semaphore("DMA_OUT") as s_out,
        nc.Block() as block,
    ):
        chunks = [chunk0, chunk1, chunk2, chunk3]

        @block.sync
        def _(sync):
            sync.dma_start(out=u_sb[:, :], in_=u_dram).then_inc(s_in, 16)
            sync.dma_start(out=v_sb[:, :], in_=v_dram).then_inc(s_in, 16)
            sync.wait_ge(s_cact, n_act_total)
            sync.wait_ge(s_cdve, n_dve_total)
            sync.dma_start(
                out=out_dram[:, 0:N_ACT], in_=act_slots[:, :]
            ).then_inc(s_out, 16)
            sync.dma_start(
                out=out_dram[:, N_ACT : N_ACT + N_DVE], in_=dve_slots[:, :]
            ).then_inc(s_out, 16)
            sync.wait_ge(s_out, 32)

        @block.tensor
        def _(tensor):
            for g in range(ntot):
                R, C, half = gsched[g]
                chunk = chunks[g % NCHUNKS]
                if g == 0:
                    tensor.wait_ge(s_in, 32)
                if g >= NCHUNKS:
                    eng, cnt = cons_count[g - NCHUNKS]
                    tensor.wait_ge(s_cact if eng == "act" else s_cdve, cnt)
                for j in range(MM_PER_UNIT):
                    jt = half * MM_PER_UNIT + j  # row-tile within super
                    mm = nc.tensor.matmul(
                        chunk[:, j * SUPER : (j + 1) * SUPER],
                        lhsT=u_sb[
                            :, R * SUPER + jt * ROWT : R * SUPER + (jt + 1) * ROWT
                        ],
                        rhs=v_sb[:, C * SUPER : (C + 1) * SUPER],
                        start=True,
                        stop=True,
                    )
                    if j == MM_PER_UNIT - 1:
                        mm.then_inc(s_prod, 1)

        @block.scalar
        def _(scalar):
            # warm the Sign activation table while the input DMA is in
            # flight (table load is ~2.7us and would otherwise serialize
            # in front of the first real chunk).
            nc.scalar.memzero(act_dummy.ap()[:, NUNITS : NUNITS + 1])
            nc.scalar.activation(
                out=act_dummy.ap()[:, NUNITS + 1 : NUNITS + 2],
                in_=act_dummy.ap()[:, NUNITS : NUNITS + 1],
                func=mybir.ActivationFunctionType.Sign,
                accum_out=act_dummy.ap()[:, NUNITS + 2 : NUNITS + 3],
            )
            for g in range(ntot):
                if gengine[g] != "act":
                    continue
                i = g % NUNITS
                chunk = chunks[g % NCHUNKS]
                scalar.wait_ge(s_prod, g + 1)
                nc.scalar.activation(
                    out=act_dummy.ap()[:, i : i + 1].broadcast_to((128, CHUNK_FD)),
                    in_=chunk[:, :],
                    func=mybir.ActivationFunctionType.Sign,
                    accum_out=act_slots[:, gslot[g] : gslot[g] + 1],
                ).then_inc(s_cact, 1)

        @block.vector
        def _(vector):
            for g in range(ntot):
                if gengine[g] != "dve":
                    continue
                i = g % NUNITS
                chunk = chunks[g % NCHUNKS]
                vector.wait_ge(s_prod, g + 1)
                nc.vector.tensor_scalar(
                    out=dve_dummy.ap()[:, i : i + 1].broadcast_to((128, CHUNK_FD)),
                    in0=chunk[:, :],
                    scalar1=0.0,
                    scalar2=None,
                    op0=mybir.AluOpType.is_gt,
                    op1=mybir.AluOpType.add,
                    accum_out=dve_slots[:, gslot[g] : gslot[g] + 1],
                ).then_inc(s_cdve, 1)

    return nc


def _prep_inputs(coords, atom_types, vdw_radii):
    """Host-side shard prep: per-batch u6/v6 [6, N] f32 arrays."""
    coords = np.asarray(coords, dtype=np.float32)  # [B, N, 3]
    atom_types = np.asarray(atom_types).astype(np.int64)  # [B, N]
    vdw_radii = np.asarray(vdw_radii, dtype=np.float32)  # [T]
    r = vdw_radii[atom_types]  # [B, N] f32 gather
    sq = np.einsum("bnd,bnd->bn", coords, coords, dtype=np.float32).astype(np.float32)
    s = (sq - r * r).astype(np.float32)
    in_maps = []
    for b in range(B):
        u = np.empty((K, N), np.float32)
        v = np.empty((K, N), np.float32)
        u[0:3] = coords[b].T
        v[0:3] = coords[b].T
        u[3] = r[b]
        v[3] = r[b]
        u[4] = -0.5 * s[b]
        v[4] = 1.0
        u[5] = 1.0
        v[5] = -0.5 * s[b]
        in_maps.append({"u6": u, "v6": v})
    return in_maps


def _combine(results):
    """Host-side gather: per-core count slots -> scalar loss."""
    chunk_elems = 128 * CHUNK_FD
    total = 0.0
    for b in range(B):
        counts = np.asarray(results[b]["counts"], np.float64)
        act = counts[:, :N_ACT].sum(axis=0)
        dve = counts[:, N_ACT:].sum(axis=0)
        cnt_b = 0.0
        for i in range(NUNITS):
            w = SCHEDULE[i // UNITS_PER_SUPER][2]
            if ENGINE[i] == "act":
                cnt = (chunk_elems + act[SLOT_IDX[i]]) / 2.0  # positives from sign-sum
            else:
                cnt = dve[SLOT_IDX[i]]
            cnt_b += w * cnt
        cnt_b -= N  # remove diagonal (G[n,n] = 2 r^2 > 0 always)
        total += (cnt_b / 2.0) / N
    return np.float32(total / B)


def kernel(coords, atom_types, vdw_radii):
    import sys

    if "/opt/trn_rl_repo" not in sys.path:
        sys.path.insert(0, "/opt/trn_rl_repo")
    from concourse.bass_utils import run_bass_kernel_spmd

    if "nc" not in _CACHE:
        _CACHE["nc"] = _build()
    nc = _CACHE["nc"]

    in_maps = _prep_inputs(coords, atom_types, vdw_radii)
    res = run_bass_kernel_spmd(nc, in_maps, core_ids=list(range(B)))
    return _combine(res.results)


if __name__ == "__main__":
    import sys

    sys.path.insert(0, "/root/problem")
    import reference as ref

    inputs = ref.setup_inputs()
    out = kernel(**{k: np.asarray(v) for k, v in inputs.items()})
    print("kernel output:", out)



# revision 2
# speedup vs baseline: 184.0086x; 184.0086x over previous
"""ClashLoss kernel for Trainium2 (8 NeuronCores, batch-parallel).

Math: for each batch b, count pairs (n, m), n != m, with
    dist(n, m) < radii[n] + radii[m]   and   dist(n, m) > EPS.
With s_n = |c_n|^2 - r_n^2, the clash condition dist^2 < (r_n + r_m)^2 is
    G[n, m] = dot(c_n, c_m) + r_n r_m - s_n/2 - s_m/2 > 0,
a 6-dim dot product u_n . v_m:
    u_n = (x, y, z, r_n, -s_n/2, 1)     (matmul stationary side)
    v_m = (x, y, z, r_m, 1, -s_m/2)     (matmul moving side)

Each core takes one batch: G = u^T v over 36 upper-triangular 512x512
super-blocks (144 matmuls of 128x512, fp32r).  ACT (Sign + accumulate)
and DVE (is_gt + accumulate) alternate counting positives per 2-bank
PSUM chunk.  Off-diagonal super-blocks weigh 2, diagonal 1; the n == m
diagonal (always G > 0) is subtracted on the host.

Built with the Tile framework (auto-scheduled semaphores).  The repeat
parameter wraps the pass in a hardware For_i loop, so repeated passes
re-execute the same small instruction stream -- this is what test.py
uses for differential timing.
"""

from contextlib import ExitStack

import numpy as np

N = 4096
B = 8
K = 6  # augmented dot-product length
SUPER = 512
ROWT = 128
NSUP = N // SUPER  # 8
EPS = 1e-8

# mm schedule: (row_offset, col_offset, weight); off-diagonal supers first
# (weight 2), then diagonal supers (weight 1).
MM_SCHED = []
for _R in range(NSUP):
    for _C in range(_R + 1, NSUP):
        for _jt in range(4):
            MM_SCHED.append((_R * SUPER + _jt * ROWT, _C * SUPER, 2))
for _R in range(NSUP):
    for _jt in range(4):
        MM_SCHED.append((_R * SUPER + _jt * ROWT, _R * SUPER, 1))
assert len(MM_SCHED) == 144

CHUNK_BANKS = 2  # PSUM banks per consumer op
N_OPS = 144 // CHUNK_BANKS
OP_ENG = ["act" if k % 2 == 0 else "dve" for k in range(N_OPS)]

_CACHE = {}


def build_program(repeat=1):
    """Build the SPMD program; repeat > 1 wraps the pass in a For_i loop."""
    import concourse.bacc as bacc
    import concourse.tile as tile
    from concourse import mybir

    nc = bacc.Bacc("TRN2", target_bir_lowering=False, debug=False)
    f32 = mybir.dt.float32
    mdt = mybir.dt.float32r
    CB = CHUNK_BANKS

    u_dram = nc.dram_tensor("u6", [K, N], mdt, kind="ExternalInput").ap()
    v_dram = nc.dram_tensor("v6", [K, N], mdt, kind="ExternalInput").ap()
    out_dram = nc.dram_tensor("counts", [128, N_OPS], f32, kind="ExternalOutput").ap()

    with ExitStack() as ctx:
        tc = ctx.enter_context(tile.TileContext(nc))
        io = ctx.enter_context(tc.tile_pool(name="io", bufs=1))
        u_sb = io.tile([K, N], mdt)
        v_sb = io.tile([K, N], mdt)
        slots = io.tile([128, N_OPS], f32)
        dummy = io.tile([128, 4], f32)
        ps = ctx.enter_context(tc.tile_pool(name="ps", bufs=1, space="PSUM"))
        pts = [
            ps.tile([128, CB * 512], f32, name=f"pt{i}") for i in range(8 // CB)
        ]

        nc.gpsimd.dma_start(u_sb[:, :], u_dram)
        nc.gpsimd.dma_start(v_sb[:, :], v_dram)

        # warm the Sign activation table while the input DMA is in flight
        nc.scalar.memzero(dummy[:, 0:1])
        nc.scalar.activation(
            out=dummy[:, 1:2],
            in_=dummy[:, 0:1],
            func=mybir.ActivationFunctionType.Sign,
            accum_out=dummy[:, 2:3],
        )

        def one_pass():
            for k in range(N_OPS):
                pt = pts[k % len(pts)]
                for j in range(CB):
                    ro, co, _w = MM_SCHED[k * CB + j]
                    nc.tensor.matmul(
                        pt[:, j * 512 : (j + 1) * 512],
                        lhsT=u_sb[:, ro : ro + ROWT],
                        rhs=v_sb[:, co : co + SUPER],
                        start=True,
                        stop=True,
                    )
                if OP_ENG[k] == "act":
                    nc.scalar.activation(
                        out=dummy[:, 0:1].broadcast_to((128, CB * 512)),
                        in_=pt[:, :],
                        func=mybir.ActivationFunctionType.Sign,
                        accum_out=slots[:, k : k + 1],
                    )
                else:
                    nc.vector.tensor_scalar(
                        out=dummy[:, 1:2].broadcast_to((128, CB * 512)),
                        in0=pt[:, :],
                        scalar1=0.0,
                        scalar2=None,
                        op0=mybir.AluOpType.is_gt,
                        op1=mybir.AluOpType.add,
                        accum_out=slots[:, k : k + 1],
                    )

        if repeat == 1:
            one_pass()
        else:
            with tc.For_i(0, repeat) as _i:
                one_pass()

        nc.gpsimd.dma_start(out_dram, slots[:, :])

    nc.compile()
    return nc


def _prep_inputs(coords, atom_types, vdw_radii):
    """Host-side shard prep: per-batch u6/v6 [6, N] f32 arrays."""
    coords = np.asarray(coords, dtype=np.float32)  # [B, N, 3]
    atom_types = np.asarray(atom_types).astype(np.int64)  # [B, N]
    vdw_radii = np.asarray(vdw_radii, dtype=np.float32)  # [T]
    r = vdw_radii[atom_types]  # [B, N]
    sq = np.einsum("bnd,bnd->bn", coords, coords, dtype=np.float32).astype(np.float32)
    s = (sq - r * r).astype(np.float32)
    in_maps = []
    for b in range(B):
        u = np.empty((K, N), np.float32)
        v = np.empty((K, N), np.float32)
        u[0:3] = coords[b].T
        v[0:3] = coords[b].T
        u[3] = r[b]
        v[3] = r[b]
        u[4] = -0.5 * s[b]
        v[4] = 1.0
        u[5] = 1.0
        v[5] = -0.5 * s[b]
        in_maps.append({"u6": u, "v6": v})
    return in_maps


def _combine(results):
    """Host-side gather: per-core count slots -> scalar loss."""
    op_elems = CHUNK_BANKS * 512 * 128
    total = 0.0
    for b in range(B):
        counts = np.asarray(results[b]["counts"], np.float64)
        col = counts.sum(axis=0)
        cnt_b = 0.0
        for k in range(N_OPS):
            w = MM_SCHED[k * CHUNK_BANKS][2]
            if OP_ENG[k] == "act":
                cnt = (op_elems + col[k]) / 2.0  # positives from sign-sum
            else:
                cnt = col[k]  # is_gt counts positives directly
            cnt_b += w * cnt
        cnt_b -= N  # remove diagonal (G[n,n] = 2 r^2 > 0 always)
        total += (cnt_b / 2.0) / N
    return np.float32(total / B)


def _run(nc, in_maps, tries=3):
    import time as _time

    from concourse.bass_utils import run_bass_kernel_spmd

    err = None
    for _ in range(tries):
        try:
            return run_bass_kernel_spmd(nc, in_maps, core_ids=list(range(B)))
        except Exception as e:  # transient tunnel errors
            err = e
            _time.sleep(2)
    raise err


def kernel(coords, atom_types, vdw_radii):
    import sys

    if "/opt/trn_rl_repo" not in sys.path:
        sys.path.insert(0, "/opt/trn_rl_repo")

    if "nc" not in _CACHE:
        _CACHE["nc"] = build_program()
    nc = _CACHE["nc"]

    in_maps = _prep_inputs(coords, atom_types, vdw_radii)
    res = _run(nc, in_maps)
    return _combine(res.results)


# revision 4
# speedup vs baseline: 192.5352x; 1.0463x over previous
"""ClashLoss kernel for Trainium2 (8 NeuronCores, batch-parallel).

Math: for each batch b, count pairs (n, m), n != m, with
    dist(n, m) < radii[n] + radii[m]   and   dist(n, m) > EPS.
With s_n = |c_n|^2 - r_n^2, the clash condition dist^2 < (r_n + r_m)^2 is
    G[n, m] = dot(c_n, c_m) + r_n r_m - s_n/2 - s_m/2 > 0,
a 6-dim dot product u_n . v_m:
    u_n = (x, y, z, r_n, -s_n/2, 1)     (matmul stationary side)
    v_m = (x, y, z, r_m, 1, -s_m/2)     (matmul moving side)

Each core takes one batch: G = u^T v over 36 upper-triangular 512x512
super-blocks (144 matmuls of 128x512, fp32r).  ACT (Sign + accumulate)
and DVE (is_gt + accumulate) alternate counting positives per 2-bank
PSUM chunk.  Off-diagonal super-blocks weigh 2, diagonal 1; the n == m
diagonal (always G > 0) is subtracted on the host.

Built with the Tile framework (auto-scheduled semaphores).  The repeat
parameter wraps the pass in a hardware For_i loop, so repeated passes
re-execute the same small instruction stream -- this is what test.py
uses for differential timing.
"""

from contextlib import ExitStack

import numpy as np

N = 4096
B = 8
K = 6  # augmented dot-product length
SUPER = 512
ROWT = 128
NSUP = N // SUPER  # 8
EPS = 1e-8

# mm schedule: (row_offset, col_offset, weight); off-diagonal supers first
# (weight 2), then diagonal supers (weight 1).
MM_SCHED = []
for _R in range(NSUP):
    for _C in range(_R + 1, NSUP):
        for _jt in range(4):
            MM_SCHED.append((_R * SUPER + _jt * ROWT, _C * SUPER, 2))
for _R in range(NSUP):
    for _jt in range(4):
        MM_SCHED.append((_R * SUPER + _jt * ROWT, _R * SUPER, 1))
assert len(MM_SCHED) == 144

CHUNK_BANKS = 2  # PSUM banks per consumer op
N_OPS = 144 // CHUNK_BANKS
OP_ENG = ["act" if k % 2 == 0 else "dve" for k in range(N_OPS)]

_CACHE = {}


def build_program(repeat=1, passes_per_iter=1):
    """Build the SPMD program; repeat > 1 wraps the pass in a For_i loop
    with passes_per_iter full passes per loop iteration."""
    import concourse.bacc as bacc
    import concourse.tile as tile
    from concourse import mybir

    nc = bacc.Bacc("TRN2", target_bir_lowering=False, debug=False)
    f32 = mybir.dt.float32
    mdt = mybir.dt.float32r
    CB = CHUNK_BANKS

    u_dram = nc.dram_tensor("u6", [K, N], mdt, kind="ExternalInput").ap()
    v_dram = nc.dram_tensor("v6", [K, N], mdt, kind="ExternalInput").ap()
    out_dram = nc.dram_tensor("counts", [128, N_OPS], f32, kind="ExternalOutput").ap()

    with ExitStack() as ctx:
        tc = ctx.enter_context(tile.TileContext(nc))
        io = ctx.enter_context(tc.tile_pool(name="io", bufs=1))
        u_sb = io.tile([K, N], mdt)
        v_sb = io.tile([K, N], mdt)
        slots = io.tile([128, N_OPS], f32)
        dummy = io.tile([128, 4], f32)
        ps = ctx.enter_context(tc.tile_pool(name="ps", bufs=1, space="PSUM"))
        pts = [
            ps.tile([128, CB * 512], f32, name=f"pt{i}") for i in range(8 // CB)
        ]

        nc.gpsimd.dma_start(u_sb[:, :], u_dram)
        nc.gpsimd.dma_start(v_sb[:, :], v_dram)

        # warm the Sign activation table while the input DMA is in flight
        nc.scalar.memzero(dummy[:, 0:1])
        nc.scalar.activation(
            out=dummy[:, 1:2],
            in_=dummy[:, 0:1],
            func=mybir.ActivationFunctionType.Sign,
            accum_out=dummy[:, 2:3],
        )

        def one_pass():
            for k in range(N_OPS):
                pt = pts[k % len(pts)]
                for j in range(CB):
                    ro, co, _w = MM_SCHED[k * CB + j]
                    nc.tensor.matmul(
                        pt[:, j * 512 : (j + 1) * 512],
                        lhsT=u_sb[:, ro : ro + ROWT],
                        rhs=v_sb[:, co : co + SUPER],
                        start=True,
                        stop=True,
                    )
                if OP_ENG[k] == "act":
                    nc.scalar.activation(
                        out=dummy[:, 0:1].broadcast_to((128, CB * 512)),
                        in_=pt[:, :],
                        func=mybir.ActivationFunctionType.Sign,
                        accum_out=slots[:, k : k + 1],
                    )
                else:
                    nc.vector.tensor_scalar(
                        out=dummy[:, 1:2].broadcast_to((128, CB * 512)),
                        in0=pt[:, :],
                        scalar1=0.0,
                        scalar2=None,
                        op0=mybir.AluOpType.is_gt,
                        op1=mybir.AluOpType.add,
                        accum_out=slots[:, k : k + 1],
                    )

        niter = repeat // passes_per_iter
        assert niter * passes_per_iter == repeat
        if niter == 1:
            for _ in range(passes_per_iter):
                one_pass()
        else:
            with tc.For_i(0, niter) as _i:
                for _ in range(passes_per_iter):
                    one_pass()

        nc.gpsimd.dma_start(out_dram, slots[:, :])

    nc.compile()
    return nc


def _prep_inputs(coords, atom_types, vdw_radii):
    """Host-side shard prep: per-batch u6/v6 [6, N] f32 arrays."""
    coords = np.asarray(coords, dtype=np.float32)  # [B, N, 3]
    atom_types = np.asarray(atom_types).astype(np.int64)  # [B, N]
    vdw_radii = np.asarray(vdw_radii, dtype=np.float32)  # [T]
    r = vdw_radii[atom_types]  # [B, N]
    sq = np.einsum("bnd,bnd->bn", coords, coords, dtype=np.float32).astype(np.float32)
    s = (sq - r * r).astype(np.float32)
    in_maps = []
    for b in range(B):
        u = np.empty((K, N), np.float32)
        v = np.empty((K, N), np.float32)
        u[0:3] = coords[b].T
        v[0:3] = coords[b].T
        u[3] = r[b]
        v[3] = r[b]
        u[4] = -0.5 * s[b]
        v[4] = 1.0
        u[5] = 1.0
        v[5] = -0.5 * s[b]
        in_maps.append({"u6": u, "v6": v})
    return in_maps


def _combine(results):
    """Host-side gather: per-core count slots -> scalar loss."""
    op_elems = CHUNK_BANKS * 512 * 128
    total = 0.0
    for b in range(B):
        counts = np.asarray(results[b]["counts"], np.float64)
        col = counts.sum(axis=0)
        cnt_b = 0.0
        for k in range(N_OPS):
            w = MM_SCHED[k * CHUNK_BANKS][2]
            if OP_ENG[k] == "act":
                cnt = (op_elems + col[k]) / 2.0  # positives from sign-sum
            else:
                cnt = col[k]  # is_gt counts positives directly
            cnt_b += w * cnt
        cnt_b -= N  # remove diagonal (G[n,n] = 2 r^2 > 0 always)
        total += (cnt_b / 2.0) / N
    return np.float32(total / B)


def _run(nc, in_maps, tries=3):
    import time as _time

    from concourse.bass_utils import run_bass_kernel_spmd

    err = None
    for _ in range(tries):
        try:
            return run_bass_kernel_spmd(nc, in_maps, core_ids=list(range(B)))
        except Exception as e:  # transient tunnel errors
            err = e
            _time.sleep(2)
    raise err


def kernel(coords, atom_types, vdw_radii):
    import sys

    if "/opt/trn_rl_repo" not in sys.path:
        sys.path.insert(0, "/opt/trn_rl_repo")

    if "nc" not in _CACHE:
        _CACHE["nc"] = build_program()
    nc = _CACHE["nc"]

    in_maps = _prep_inputs(coords, atom_types, vdw_radii)
    res = _run(nc, in_maps)
    return _combine(res.results)
